# revision 2
# baseline (speedup 1.0000x reference)
"""GATv2 (3-layer, DGL-style, share_weights, elu) on 8 Trainium2 NeuronCores.

v5 (on top of v4):
  - Batched Prelu: per scw-block the 2*scw x-matmuls write slices of ONE
    wide PSUM tile; a single Prelu drains it (ACT per-instr overhead is
    ~352 cycles, so fewer+wider ACT ops).
  - Exp written by ACT directly into the msg tile's denominator columns
    (kills the DVE tensor_copy).
  - Finalize normalization on DVE as one broadcast-mult; h stays
    head-interleaved END-TO-END (W1/W2 rows pre-permuted on host; final
    per-graph sums de-interleaved on host in finish()). Kills the 4 ACT
    de-interleave copies per tile.
  - cfg gen_m2: build the M2 scatter one-hot on-device per chunk via
    DVE is_equal(iotaRP, dloc) instead of streaming it from HBM
    (saves ~22.5MB/layer/core of HBM traffic).

v4: dma_gather fs + PE-reconstructed fd edge phase.
  - feat[dst] is NOT gathered: dst rows are tile-local, so fd (and the
    x = fs + fd sum) is built on the tensor engine per 128-edge chunk as
    M1^T @ nb + I @ fs into rotating PSUM tiles (M1 = host-built
    transposed one-hots, nb = the group's node block). Halves SWDGE
    gather rows and removes the DVE x-add.
  - Score: lr = Prelu(x, 0.2) on ACT (HW honors alpha for Prelu), then
    score = sum_d a_d * lr_d  via DVE mult + 2-step reduce. No aux columns:
    gather rows are exactly 256 bf16 = 512B (dma_gather granularity).
  - Gathers: gpsimd.dma_gather batches a whole supertile of tiles per
    instruction (SWDGE fixed ~1us amortized over thousands of rows).
    int16 index limit (<=32767) handled by splitting the gather table:
    fs from feat_full[0:25088] / feat_full[25088:], fd from the local
    shard ag_in[l] (dst rows are core-local by the node partition).
    Per (core,tile) the edge list is ordered [lo-src | pad | hi-src | pad]
    with 128-aligned per-class caps (max over cores, SPMD-static); pads
    gather row 0 and carry zero one-hot columns.
  - Aggregation per 128-edge chunk: one PE matmul with host-built one-hot
    lhsT (M2) and rhs [alpha*fs (interleaved) | exp(score) per head].
  - Feature dims head-interleaved (col = d*H + h) so the alpha broadcast
    in the message multiply is innermost-contiguous (DVE 2x mode);
    de-interleaved for free in the finalize access pattern.
  - Projections: lhsT via XBAR dma_start_transpose from bf16 h_dram
    (layer 0: host-pretransposed featT). bf16 everywhere off-PSUM.
"""

import numpy as np

import sys

sys.path.insert(0, "/opt/trn_rl_repo")

from concourse import bacc, bass, mybir, tile  # noqa: E402

f32 = mybir.dt.float32
bf16 = mybir.dt.bfloat16
i32 = mybir.dt.int32
i16 = mybir.dt.int16
AF = mybir.ActivationFunctionType
ALU = mybir.AluOpType
P = 128

N_NODES = 50000
N_EDGES = 600000
HEADS = 4
DH = 64
D = HEADS * DH  # 256
DE = D + HEADS  # msg width incl exp cols
IN0 = 128
NUM_GRAPHS = 8
N_CORES = 8
NPC = N_NODES // N_CORES  # 6250
TILES = (NPC + P - 1) // P  # 49
NPCP = TILES * P  # 6272 padded
NTOTP = N_CORES * NPCP  # 50176
HALF = NTOTP // 2  # 25088 rows per gather table


def _build_nc(cfg):
    capL = cfg["capL"]  # per tile, chunks of lo-src edges (128-mult slots/128)
    capH = cfg["capH"]
    T = TILES
    K = [capL[t] + capH[t] for t in range(T)]
    offs = np.concatenate([[0], np.cumsum(K)]).astype(int)
    SCW = int(cfg.get("scw", 4))
    ST = int(cfg.get("supertile", 4))
    G = NUM_GRAPHS

    egroups = [list(range(g, min(g + ST, T))) for g in range(0, T, ST)]
    # per-group gather sizes (in chunks)
    grpL = [sum(capL[t] for t in grp) for grp in egroups]
    grpH = [sum(capH[t] for t in grp) for grp in egroups]
    grpK = [sum(K[t] for t in grp) for grp in egroups]
    MAXL, MAXH, MAXK = max(grpL), max(grpH), max(grpK)
    # idx param column offsets (in i16 cols = slots/16)
    offL16 = np.concatenate([[0], np.cumsum([x * 8 for x in grpL])]).astype(int)
    offH16 = np.concatenate([[0], np.cumsum([x * 8 for x in grpH])]).astype(int)
    offK16 = np.concatenate([[0], np.cumsum([x * 8 for x in grpK])]).astype(int)

    GCAP = int(cfg.get("gcap", 12))  # chunks per dma_gather piece
    NQ = int(cfg.get("nq", 1))
    qctr = [0]

    def nextq():
        qctr[0] = (qctr[0] + 1) % NQ
        return qctr[0]
    nc = bacc.Bacc(
        "TRN2",
        target_bir_lowering=False,
        debug=False,
        dynamic_dma_scratch_size=int(cfg.get("dma_scratch", 32768)),
        num_swdge_queues=int(cfg.get("nq", 1)),
    )

    featTp = nc.declare_dram_parameter("featT", [P, NPCP], bf16, isOutput=False)
    Wp = [
        nc.declare_dram_parameter(
            f"W{l}a", [IN0 if l == 0 else D, D], bf16, isOutput=False
        )
        for l in range(3)
    ]
    awp = [
        nc.declare_dram_parameter(f"aw{l}", [P, SCW * D], bf16, isOutput=False)
        for l in range(3)
    ]
    gxLp = nc.declare_dram_parameter("gxL", [P, int(offL16[-1])], i16, isOutput=False)
    gxHp = nc.declare_dram_parameter("gxH", [P, int(offH16[-1])], i16, isOutput=False)
    gen_m2 = cfg.get("gen_m2", True)
    if gen_m2:
        dlocCp = nc.declare_dram_parameter(
            "dlocC", [P, int(offs[-1])], bf16, isOutput=False
        )
        iotaRPp = nc.declare_dram_parameter("iotaRP", [P, P], bf16, isOutput=False)
    else:
        m2p = nc.declare_dram_parameter(
            "m2", [P, int(offs[-1]) * P], bf16, isOutput=False
        )
    m1p = nc.declare_dram_parameter("m1", [P, int(offs[-1]) * P], bf16, isOutput=False)
    idenp = nc.declare_dram_parameter("iden", [P, P], bf16, isOutput=False)
    gohp = nc.declare_dram_parameter("goh", [P, T * G], bf16, isOutput=False)
    outp = nc.declare_dram_parameter("gsum", [G, D], f32, isOutput=True)
    debug = cfg.get("debug", False)
    if debug:
        d_ag0 = nc.declare_dram_parameter("d_ag0", [NPCP, D], bf16, isOutput=True)
        d_ff0 = nc.declare_dram_parameter("d_ff0", [NTOTP, D], bf16, isOutput=True)
        d_h1 = nc.declare_dram_parameter("d_h1", [NPCP, D], bf16, isOutput=True)

    h_dram = [None, nc.dram_tensor("h1", [NPCP, D], bf16),
              nc.dram_tensor("h2", [NPCP, D], bf16)]
    ag_in = [nc.dram_tensor(f"agin{l}", [NPCP, D], bf16) for l in range(3)]
    feat_full = [
        nc.dram_tensor(f"ff{l}", [NTOTP, D], bf16, addr_space="Shared")
        for l in range(3)
    ]
    rg = [list(range(N_CORES))]

    with tile.TileContext(nc) as tc:
        with (
            tc.tile_pool(name="consts", bufs=1) as cp,
            tc.tile_pool(name="gath", bufs=cfg.get("gp_bufs", 2)) as gp,
            tc.tile_pool(name="edge", bufs=cfg.get("ep_bufs", 2)) as ep,
            tc.tile_pool(name="small", bufs=cfg.get("sp_bufs", 3)) as sp,
            tc.tile_pool(name="node", bufs=cfg.get("np_bufs", 2)) as npo,
            tc.tile_pool(name="ps", bufs=2, space="PSUM") as pp,
        ):
            aw_t = []
            for l in range(3):
                t_ = cp.tile([P, SCW * D], bf16, tag=f"aw{l}")
                nc.sync.dma_start(t_[:], awp[l][:])
                aw_t.append(t_)
            W_t = []
            for l in range(3):
                ind = IN0 if l == 0 else D
                chunks = []
                for k in range(ind // P):
                    wt = cp.tile([P, D], bf16, tag=f"W{l}_{k}")
                    nc.sync.dma_start(wt[:], Wp[l][k * P : (k + 1) * P, :])
                    chunks.append(wt)
                W_t.append(chunks)
            goh_t = cp.tile([P, T * G], bf16, tag="goh")
            nc.sync.dma_start(goh_t[:], gohp[:])
            gacc = cp.tile([G, D], f32, tag="gacc")
            nc.vector.memset(gacc[:], 0.0)
            iden_t = cp.tile([P, P], bf16, tag="iden")
            nc.sync.dma_start(iden_t[:], idenp[:])
            if gen_m2:
                iotaRP_t = cp.tile([P, P], bf16, tag="iotaRP")
                nc.sync.dma_start(iotaRP_t[:], iotaRPp[:])
                dlocC_t = cp.tile([P, int(offs[-1])], bf16, tag="dlocC")
                nc.sync.dma_start(dlocC_t[:], dlocCp[:])

            for l in range(3):
                ind = IN0 if l == 0 else D
                nk = ind // P
                # ---------------- projection ----------------
                for grp in egroups:
                    nt = len(grp)
                    g0 = grp[0]
                    rows0 = g0 * P
                    hT = npo.tile([P, ST * 2 * P], bf16, tag="hT")
                    if l == 0:
                        nc.sync.dma_start(
                            hT[:, : nt * P], featTp[:, rows0 : rows0 + nt * P]
                        )
                    else:
                        for gi, t in enumerate(grp):
                            for k in range(nk):
                                nc.sync.dma_start_transpose(
                                    hT[:, (gi * nk + k) * P : (gi * nk + k + 1) * P],
                                    h_dram[l][t * P : (t + 1) * P, k * P : (k + 1) * P],
                                )
                    proj_st = npo.tile([P, ST * D], bf16, tag="proj_s")
                    for gi, t in enumerate(grp):
                        ps_proj = pp.tile(
                            [P, DE], f32, tag="agg0", space="PSUM", bufs=1,
                            name="ps_proj",
                        )
                        for k in range(nk):
                            nc.tensor.matmul(
                                ps_proj[:, :D],
                                lhsT=hT[:, (gi * nk + k) * P : (gi * nk + k + 1) * P],
                                rhs=W_t[l][k][:],
                                start=(k == 0),
                                stop=(k == nk - 1),
                            )
                        nc.scalar.copy(
                            proj_st[:, gi * D : (gi + 1) * D], ps_proj[:, :D]
                        )
                    if nt > 1:
                        nc.sync.dma_start(
                            ag_in[l][rows0 : rows0 + nt * P, :].rearrange(
                                "(a p) d -> p a d", p=P
                            ),
                            proj_st[:, : nt * D].rearrange("p (a d) -> p a d", d=D),
                        )
                    else:
                        nc.sync.dma_start(
                            ag_in[l][rows0 : rows0 + P, :], proj_st[:, :D]
                        )
                # ---------------- replicate ----------------
                if cfg.get("sim_no_collective"):
                    nc.sync.dma_start(feat_full[l][0:NPCP, :], ag_in[l][:])
                else:
                    nc.gpsimd.collective_compute(
                        "AllGather",
                        ALU.bypass,
                        ins=[ag_in[l][:]],
                        outs=[feat_full[l][:]],
                        replica_groups=rg,
                    )
                if debug and l == 0:
                    nc.sync.dma_start(d_ag0[:], ag_in[0][:])
                    nc.sync.dma_start(d_ff0[:], feat_full[0][:])
                # ---------------- edge phase ----------------
                for gidx, grp in enumerate(egroups):
                  nt = len(grp)
                  g0 = grp[0]
                  nL, nH, nK = grpL[gidx], grpH[gidx], grpK[gidx]
                  # group gathers: fs lo / fs hi / fd local
                  ixL = sp.tile([P, MAXL * 8], i16, tag="ixL")
                  nc.sync.dma_start(
                      ixL[:, : nL * 8], gxLp[:, int(offL16[gidx]) : int(offL16[gidx]) + nL * 8]
                  )
                  fsL = gp.tile([P, MAXL * D], bf16, tag="fsL")
                  for p0 in range(0, nL, GCAP):
                      p1 = min(p0 + GCAP, nL)
                      if cfg.get("ablate_gather"):
                          nc.sync.dma_start(
                              fsL[:, p0 * D : p1 * D].rearrange(
                                  "p (b e) -> p b e", e=D
                              ),
                              feat_full[l][0 : (p1 - p0) * P, :].rearrange(
                                  "(b p) d -> p b d", p=P
                              ),
                          )
                          continue
                      nc.gpsimd.dma_gather(
                          out_ap=fsL[:, p0 * D : p1 * D].rearrange(
                              "p (b e) -> p b e", e=D
                          ),
                          in_ap=feat_full[l][0:HALF, :],
                          idxs_ap=ixL[:, p0 * 8 : p1 * 8],
                          num_idxs=(p1 - p0) * P,
                          num_idxs_reg=(p1 - p0) * P,
                          elem_size=D,
                          queue_num=nextq(),
                      )
                  ixH = sp.tile([P, max(MAXH, 1) * 8], i16, tag="ixH")
                  fsH = gp.tile([P, max(MAXH, 1) * D], bf16, tag="fsH")
                  if nH > 0:
                      nc.sync.dma_start(
                          ixH[:, : nH * 8],
                          gxHp[:, int(offH16[gidx]) : int(offH16[gidx]) + nH * 8],
                      )
                      for p0 in range(0, nH, GCAP):
                          p1 = min(p0 + GCAP, nH)
                          if cfg.get("ablate_gather"):
                              nc.sync.dma_start(
                                  fsH[:, p0 * D : p1 * D].rearrange(
                                      "p (b e) -> p b e", e=D
                                  ),
                                  feat_full[l][0 : (p1 - p0) * P, :].rearrange(
                                      "(b p) d -> p b d", p=P
                                  ),
                              )
                              continue
                          nc.gpsimd.dma_gather(
                              out_ap=fsH[:, p0 * D : p1 * D].rearrange(
                                  "p (b e) -> p b e", e=D
                              ),
                              in_ap=feat_full[l][HALF:NTOTP, :],
                              idxs_ap=ixH[:, p0 * 8 : p1 * 8],
                              num_idxs=(p1 - p0) * P,
                              num_idxs_reg=(p1 - p0) * P,
                              elem_size=D,
                              queue_num=nextq(),
                          )
                  nb_g = npo.tile([P, ST * D], bf16, tag="nb")
                  if nt > 1:
                      nc.sync.dma_start(
                          nb_g[:, : nt * D].rearrange("p (a d) -> p a d", d=D),
                          ag_in[l][g0 * P : g0 * P + nt * P, :].rearrange(
                              "(a p) d -> p a d", p=P
                          ),
                      )
                  else:
                      nc.sync.dma_start(
                          nb_g[:, :D], ag_in[l][g0 * P : g0 * P + P, :]
                      )
                  m1t = gp.tile([P, MAXK * P], bf16, tag="m1")
                  nc.sync.dma_start(
                      m1t[:, : nK * P],
                      m1p[:, int(offs[g0]) * P : (int(offs[g0]) + nK) * P],
                  )
                  m2t = gp.tile([P, MAXK * P], bf16, tag="m2")
                  if gen_m2:
                      # m2[e, loc] = (dloc[e] == loc), one TT per group:
                      # in0 = iota row replicated down partitions (const),
                      # in1 = per-chunk dloc column broadcast along loc.
                      nc.vector.tensor_tensor(
                          out=m2t[:, : nK * P].rearrange(
                              "p (k e) -> p k e", e=P
                          ),
                          in0=iotaRP_t[:, :].rearrange(
                              "p (o e) -> p o e", o=1
                          ).to_broadcast([P, nK, P]),
                          in1=dlocC_t[
                              :, int(offs[g0]) : int(offs[g0]) + nK
                          ].rearrange("p (k o) -> p k o", o=1).to_broadcast(
                              [P, nK, P]
                          ),
                          op=ALU.is_equal,
                      )
                  else:
                      nc.sync.dma_start(
                          m2t[:, : nK * P],
                          m2p[:, int(offs[g0]) * P : (int(offs[g0]) + nK) * P],
                      )
                  out_st = npo.tile([P, ST * D], bf16, tag="out_t")
                  # class-major stream order: q in [0,nL) -> fsL chunk q;
                  # q in [nL,nK) -> fsH chunk q-nL. fd/m2 are host-ordered
                  # the same way. Map stream pos -> (tile slot gi).
                  q2gi = []
                  for gi, t in enumerate(grp):
                      q2gi += [gi] * capL[t]
                  for gi, t in enumerate(grp):
                      q2gi += [gi] * capH[t]
                  seen = [0] * nt
                  ps_aggs = []
                  for gi in range(nt):
                      ps_agg_i = pp.tile(
                          [P, DE], f32, tag=f"agg{gi}", space="PSUM", bufs=1,
                          name=f"ps_agg{gi}",
                      )
                      ps_aggs.append(ps_agg_i)
                  q = 0
                  while q < nK:
                      m = min(SCW, nK - q)
                      if q < nL:
                          m = min(m, nL - q)  # don't cross the lo/hi boundary
                      WF = m * D
                      WW = m * DE
                      fs_v = (
                          fsL[:, q * D : (q + m) * D]
                          if q < nL
                          else fsH[:, (q - nL) * D : (q - nL + m) * D]
                      )
                      xb = pp.tile(
                          [P, SCW * D], f32, tag="xb", space="PSUM",
                          bufs=cfg.get("xbufs", 2),
                      )
                      for j in range(m):
                          gi_j = q2gi[q + j]
                          nc.tensor.matmul(
                              xb[:, j * D : (j + 1) * D],
                              lhsT=m1t[:, (q + j) * P : (q + j + 1) * P],
                              rhs=nb_g[:, gi_j * D : (gi_j + 1) * D],
                              start=True, stop=False,
                          )
                          nc.tensor.matmul(
                              xb[:, j * D : (j + 1) * D],
                              lhsT=iden_t[:],
                              rhs=fs_v[:, j * D : (j + 1) * D],
                              start=False, stop=True,
                          )
                      lr = ep.tile([P, SCW * D], bf16, tag="lr")
                      nc.scalar.activation(
                          lr[:, :WF], xb[:, :WF], AF.Prelu, alpha=0.2,
                      )
                      mm = ep.tile([P, SCW * D], bf16, tag="mm")
                      nc.vector.tensor_tensor(
                          out=mm[:, :WF], in0=lr[:, :WF],
                          in1=aw_t[l][:, :WF], op=ALU.mult,
                      )
                      u = sp.tile([P, SCW * HEADS], f32, tag="u")
                      m2h = ep.tile([P, SCW * D // 2], bf16, tag="mh")
                      nc.vector.tensor_tensor(
                          out=m2h[:, : WF // 2],
                          in0=mm[:, :WF].rearrange("p (g d) -> p g d", d=D)[
                              :, :, 0 : D // 2
                          ],
                          in1=mm[:, :WF].rearrange("p (g d) -> p g d", d=D)[
                              :, :, D // 2 : D
                          ],
                          op=ALU.add,
                      )
                      m4h = ep.tile([P, SCW * D // 4], bf16, tag="m4h")
                      nc.vector.tensor_tensor(
                          out=m4h[:, : WF // 4],
                          in0=m2h[:, : WF // 2].rearrange(
                              "p (g d) -> p g d", d=D // 2
                          )[:, :, 0 : D // 4],
                          in1=m2h[:, : WF // 2].rearrange(
                              "p (g d) -> p g d", d=D // 2
                          )[:, :, D // 4 : D // 2],
                          op=ALU.add,
                      )
                      m8h = ep.tile([P, SCW * D // 8], bf16, tag="m8h")
                      nc.vector.tensor_tensor(
                          out=m8h[:, : WF // 8],
                          in0=m4h[:, : WF // 4].rearrange(
                              "p (g d) -> p g d", d=D // 4
                          )[:, :, 0 : D // 8],
                          in1=m4h[:, : WF // 4].rearrange(
                              "p (g d) -> p g d", d=D // 4
                          )[:, :, D // 8 : D // 4],
                          op=ALU.add,
                      )
                      nc.vector.reduce_sum(
                          out=u[:, : m * HEADS],
                          in_=m8h[:, : WF // 8].rearrange(
                              "p (g d h) -> p g h d", h=HEADS, d=DH // 8
                          ),
                          axis=mybir.AxisListType.X,
                      )
                      msg = ep.tile([P, SCW * DE], bf16, tag="msg")
                      msg3 = msg[:, :WW].rearrange("p (g w) -> p g w", w=DE)
                      nc.scalar.activation(
                          msg3[:, :, D:DE],
                          u[:, : m * HEADS].rearrange(
                              "p (g h) -> p g h", h=HEADS
                          ),
                          AF.Exp,
                      )
                      nc.vector.tensor_tensor(
                          out=msg3[:, :, 0:D].rearrange(
                              "p g (d h) -> p g d h", h=HEADS
                          ),
                          in0=fs_v.rearrange("p (g w) -> p g w", w=D)
                          .rearrange("p g (d h) -> p g d h", h=HEADS),
                          in1=msg[:, :WW]
                          .rearrange("p (g o w) -> p g o w", o=1, w=DE)[
                              :, :, :, D:DE
                          ]
                          .to_broadcast([P, m, DH, HEADS]),
                          op=ALU.mult,
                      )
                      for j in range(m):
                          gi = q2gi[q + j]
                          t = grp[gi]
                          kt = K[t]
                          nc.tensor.matmul(
                              ps_aggs[gi][:],
                              lhsT=m2t[:, (q + j) * P : (q + j + 1) * P],
                              rhs=msg[:, j * DE : (j + 1) * DE],
                              start=(seen[gi] == 0),
                              stop=(seen[gi] == kt - 1),
                          )
                          seen[gi] += 1
                          if seen[gi] == kt:
                              # normalize; stays head-interleaved
                              den = sp.tile([P, HEADS], f32, tag="den")
                              nc.vector.tensor_scalar_max(
                                  den[:], ps_aggs[gi][:, D:DE], 1e-30
                              )
                              rcp = sp.tile([P, HEADS], f32, tag="rcp")
                              nc.vector.reciprocal(rcp[:], den[:])
                              oD = gi * D
                              if cfg.get("norm_act", True):
                                  # 4 strided per-head ACT copies: DVE is
                                  # the bottleneck engine, ACT has slack
                                  for hh in range(HEADS):
                                      nc.scalar.activation(
                                          out_st[:, oD : oD + D].rearrange(
                                              "p (d h) -> p h d", h=HEADS
                                          )[:, hh, :],
                                          ps_aggs[gi][:, 0:D].rearrange(
                                              "p (d h) -> p h d", h=HEADS
                                          )[:, hh, :],
                                          AF.Copy,
                                          scale=rcp[:, hh : hh + 1],
                                      )
                              else:
                                  nc.vector.tensor_tensor(
                                      out=out_st[:, oD : oD + D].rearrange(
                                          "p (d h) -> p d h", h=HEADS
                                      ),
                                      in0=ps_aggs[gi][:, 0:D].rearrange(
                                          "p (d h) -> p d h", h=HEADS
                                      ),
                                      in1=rcp[:, :]
                                      .rearrange("p (o h) -> p o h", o=1)
                                      .to_broadcast([P, DH, HEADS]),
                                      op=ALU.mult,
                                  )
                      q += m
                  # ---------------- finalize (residual + elu) ----------------
                  rows0 = g0 * P
                  W_g = nt * D
                  if l > 0:
                      hres = npo.tile([P, ST * D], bf16, tag="hres")
                      if nt > 1:
                          nc.sync.dma_start(
                              hres[:, :W_g].rearrange("p (a d) -> p a d", d=D),
                              h_dram[l][rows0 : rows0 + nt * P, :].rearrange(
                                  "(a p) d -> p a d", p=P
                              ),
                          )
                      else:
                          nc.sync.dma_start(
                              hres[:, :D], h_dram[l][rows0 : rows0 + P, :]
                          )
                      nc.vector.tensor_tensor(
                          out=out_st[:, :W_g], in0=out_st[:, :W_g],
                          in1=hres[:, :W_g], op=ALU.add,
                      )
                  # elu(x) = exp(min(x,0)) + max(x,0) - 1
                  mneg = npo.tile([P, ST * D], bf16, tag="mneg")
                  nc.vector.tensor_scalar_min(mneg[:, :W_g], out_st[:, :W_g], 0.0)
                  epos = npo.tile([P, ST * D], bf16, tag="epos")
                  nc.vector.tensor_scalar(
                      out=epos[:, :W_g], in0=out_st[:, :W_g],
                      scalar1=0.0, scalar2=-1.0, op0=ALU.max, op1=ALU.add,
                  )
                  eneg = npo.tile([P, ST * D], bf16, tag="eneg")
                  nc.scalar.activation(eneg[:, :W_g], mneg[:, :W_g], AF.Exp)
                  hn = npo.tile([P, ST * D], bf16, tag="hn")
                  nc.vector.tensor_tensor(
                      out=hn[:, :W_g], in0=eneg[:, :W_g], in1=epos[:, :W_g],
                      op=ALU.add,
                  )
                  if l < 2:
                      if nt > 1:
                          nc.sync.dma_start(
                              h_dram[l + 1][rows0 : rows0 + nt * P, :].rearrange(
                                  "(a p) d -> p a d", p=P
                              ),
                              hn[:, :W_g].rearrange("p (a d) -> p a d", d=D),
                          )
                      else:
                          nc.sync.dma_start(
                              h_dram[l + 1][rows0 : rows0 + P, :], hn[:, :D]
                          )
                      if debug and l == 0 and g0 + nt == T:
                          nc.sync.dma_start(d_h1[:], h_dram[1][:])
                  else:
                      for gi, t in enumerate(grp):
                          ps_g = pp.tile(
                              [P, DE], f32, tag="agg1", space="PSUM", bufs=1,
                              name="ps_g",
                          )
                          nc.tensor.matmul(
                              ps_g[0:G, 0:D],
                              lhsT=goh_t[:, t * G : (t + 1) * G],
                              rhs=hn[:, gi * D : (gi + 1) * D],
                              start=True, stop=True,
                          )
                          nc.vector.tensor_tensor(
                              out=gacc[:], in0=gacc[:], in1=ps_g[0:G, 0:D],
                              op=ALU.add,
                          )
            nc.sync.dma_start(outp[:], gacc[:])
    nc.compile()
    return nc


def _preprocess(src, dst, graph_ids, st=2):
    order = np.argsort(dst, kind="stable")
    src_s = src[order].astype(np.int64)
    dst_s = dst[order].astype(np.int64)
    srow = src_s + (NPCP - NPC) * (src_s // NPC)  # padded global row

    node_bounds = []
    for c in range(N_CORES):
        for t in range(TILES):
            node_bounds.append(min(c * NPC + t * P, (c + 1) * NPC))
    node_bounds.append(N_NODES)
    bounds = np.searchsorted(dst_s, np.asarray(node_bounds))

    # per (core, tile): lo/hi edge lists
    lo_lists = {}
    hi_lists = {}
    nlo = np.zeros((N_CORES, TILES), int)
    nhi = np.zeros((N_CORES, TILES), int)
    for c in range(N_CORES):
        for t in range(TILES):
            e0, e1 = bounds[c * TILES + t], bounds[c * TILES + t + 1]
            sr = srow[e0:e1]
            dloc = (dst_s[e0:e1] - c * NPC).astype(np.int64)
            lo = sr < HALF
            lo_lists[(c, t)] = (sr[lo], dloc[lo])
            hi_lists[(c, t)] = (sr[~lo] - HALF, dloc[~lo])
            nlo[c, t] = int(lo.sum())
            nhi[c, t] = int((~lo).sum())

    def rup(x):
        return int(-(-x // P))

    capL = [max(1, rup(int(nlo[:, t].max()))) for t in range(TILES)]
    capH = [rup(int(nhi[:, t].max())) for t in range(TILES)]
    K = [capL[t] + capH[t] for t in range(TILES)]
    offs = np.concatenate([[0], np.cumsum(K)]).astype(int)
    TC = int(offs[-1])

    egroups = [list(range(g, min(g + st, TILES))) for g in range(0, TILES, st)]
    grpL = [sum(capL[t] for t in grp) for grp in egroups]
    grpH = [sum(capH[t] for t in grp) for grp in egroups]
    grpK = [sum(K[t] for t in grp) for grp in egroups]

    m2 = np.zeros((N_CORES, P, TC * P), np.float32)
    m1 = np.zeros((N_CORES, P, TC * P), np.float32)
    dlocC = np.full((N_CORES, P, TC), -1.0, np.float32)
    CL = sum(grpL) * 8
    CH = sum(grpH) * 8
    gxL = np.zeros((N_CORES, P, CL), np.int16)
    gxH = np.zeros((N_CORES, P, CH), np.int16)

    def put_wrapped(dstarr, c, colbase, idx_list):
        # wrapped int16 layout: idx i -> [i%16, colbase + i//16], replicated
        # to all 8 partition groups of 16
        n = len(idx_list)
        if n == 0:
            return
        i = np.arange(n)
        for g in range(8):
            dstarr[c, g * 16 + (i % 16), colbase + i // 16] = idx_list

    oL = oH = oK = 0
    for gidx, grp in enumerate(egroups):
        g0 = grp[0]
        goff = int(offs[g0])  # group's global chunk base (m2 columns)
        for c in range(N_CORES):
            bL = 0
            bH = sum(capL[t] for t in grp)  # hi stream starts after all lo
            for t in grp:
                slo, dlo = lo_lists[(c, t)]
                shi, dhi = hi_lists[(c, t)]
                # fs idx lists (class-major, within-group chunk bases)
                il = np.zeros(capL[t] * P, np.int16)
                il[: len(slo)] = slo
                put_wrapped(gxL, c, oL + bL * 8, il)
                if capH[t]:
                    ih = np.zeros(capH[t] * P, np.int16)
                    ih[: len(shi)] = shi
                    put_wrapped(gxH, c, oH + (bH - sum(capL[t2] for t2 in grp)) * 8, ih)
                # fd idxs (group-local rows) + M2 one-hots, stream order
                for sbase, (sl, dl) in (
                    (bL, (slo, dlo)),
                    (bH, (shi, dhi)),
                ):
                    if len(dl) == 0:
                        continue
                    j = np.arange(len(dl))
                    loc = dl - t * P
                    m2[c, j % P, (goff + sbase + j // P) * P + loc] = 1.0
                    m1[c, loc, (goff + sbase + j // P) * P + (j % P)] = 1.0
                    dlocC[c, j % P, goff + sbase + j // P] = loc
                bL += capL[t]
                bH += capH[t]
        oL += grpL[gidx] * 8
        oH += grpH[gidx] * 8

    goh = np.zeros((N_CORES, P, TILES * NUM_GRAPHS), np.float32)
    for c in range(N_CORES):
        for t in range(TILES):
            lo = c * NPC + t * P
            hi = min(lo + P, (c + 1) * NPC)
            ids = graph_ids[lo:hi]
            goh[c, np.arange(hi - lo), t * NUM_GRAPHS + ids] = 1.0

    return {
        "capL": capL, "capH": capH,
        "gxL": gxL, "gxH": gxH,
        "m2": m2, "m1": m1, "dlocC": dlocC, "goh": goh, "st": st,
    }


ILV_IDX = np.arange(D).reshape(HEADS, DH).T.reshape(-1)


def _interleave(Wcols):
    return Wcols[..., ILV_IDX]


def _make_in_maps(inputs, pre, cfg):
    import ml_dtypes

    SCW = int(cfg.get("scw", 4))

    def b(x):
        return np.ascontiguousarray(x).astype(ml_dtypes.bfloat16)

    feat = np.asarray(inputs["feat"], np.float32)
    a_l = [np.asarray(inputs[f"a{l}"], np.float32) for l in range(3)]
    W_l = [np.asarray(inputs[f"W{l}"], np.float32) for l in range(3)]

    Wi = [_interleave(W) for W in W_l]
    # h is head-interleaved end-to-end: W1/W2 consume interleaved rows
    Wi[1] = Wi[1][ILV_IDX, :]
    Wi[2] = Wi[2][ILV_IDX, :]
    aws = [np.tile(_interleave(a.reshape(1, D)), (P, SCW)) for a in a_l]

    featT = np.zeros((N_CORES, P, NPCP), np.float32)
    for c in range(N_CORES):
        featT[c, :, :NPC] = feat[c * NPC : (c + 1) * NPC].T

    in_maps = []
    for c in range(N_CORES):
        in_maps.append(
            {
                "featT": b(featT[c]),
                "W0a": b(Wi[0]),
                "W1a": b(Wi[1]),
                "W2a": b(Wi[2]),
                "aw0": b(aws[0]),
                "aw1": b(aws[1]),
                "aw2": b(aws[2]),
                "gxL": np.ascontiguousarray(pre["gxL"][c]),
                "gxH": np.ascontiguousarray(pre["gxH"][c]),
                "m2": b(pre["m2"][c]),
                "m1": b(pre["m1"][c]),
                "dlocC": b(pre["dlocC"][c]),
                "iotaRP": b(np.tile(np.arange(P, dtype=np.float32), (P, 1))),
                "iden": b(np.eye(P, dtype=np.float32)),
                "goh": b(pre["goh"][c]),
            }
        )
    return in_maps


def _full_cfg(pre):
    return {
        "capL": pre["capL"],
        "capH": pre["capH"],
        "scw": 6,
        "xbufs": 2,
        "gen_m2": False,
        "supertile": pre["st"],
        # proven-on-HW gather config: 6-chunk pieces keep total in-flight
        # SWDGE descriptors under the 4096-slot ring; 4 queues parallelize
        # the Q7 descriptor-generation ucode (measured 6.05 -> 3.36 ms).
        "gcap": 6,
        "dma_scratch": 65536,
        "nq": 4,
    }


def build_for_test(inputs, cfg_over=None):
    src = np.asarray(inputs["src"], np.int32)
    dst = np.asarray(inputs["dst"], np.int32)
    graph_ids = np.asarray(inputs["graph_ids"], np.int32)
    st = (cfg_over or {}).get("supertile", 2)
    pre = _preprocess(src, dst, graph_ids, st=st)
    cfg = _full_cfg(pre)
    if cfg_over:
        cfg.update(cfg_over)
    nc = _build_nc(cfg)
    in_maps = _make_in_maps(inputs, pre, cfg)
    counts = np.bincount(graph_ids, minlength=NUM_GRAPHS).astype(np.float32)

    def finish(results):
        total = np.zeros((NUM_GRAPHS, D), np.float32)
        for r in results:
            total += np.asarray(r["gsum"], np.float32)
        out = np.empty_like(total)
        out[:, ILV_IDX] = total  # de-interleave heads
        return (out / np.maximum(counts, 1.0)[:, None]).astype(np.float32)

    return nc, in_maps, finish


def kernel(**inputs):
    from concourse.bass_utils import run_bass_kernel_spmd

    nc, in_maps, finish = build_for_test(inputs)
    res = run_bass_kernel_spmd(nc, in_maps, list(range(N_CORES)))
    return finish(res.results)

